# revision 30
# baseline (speedup 1.0000x reference)
"""Trainium2 Bass kernel for nn_CNN_84241488544497.

The reference network collapses algebraically:
  - `_row` is identically zero (exp(-d^2/2e-4) underflows to 0.0 in fp32).
  - x is an exact 0/1 one-hot, so nz == xp and the `_column` scatter is
    xp_new = x @ M with M = I + V, V a 20x20 matrix built from lpm/pm.
  - The 9 conv+avgpool stages form one linear map T (512x8) per row.
  => out[b] = M^T @ (x[b]^T @ T)  with M (20,20), T (512,8) host-folded.

Device kernel (per core, 64 batches, pure data parallel over B=512):
  stage 1: Gt[c, (b,i)] = sum_p T[p,c] * x[p,(b,i)] via fp8e4m3
           DoubleRow matmuls: T scaled by 2^16 and split hi/lo in fp8
           (T values ~2e-3 underflow e4m3 otherwise); x one-hot is
           exact in fp8.  Each DoubleRow instruction contracts 2 K-
           subtiles (chunk pairs for hi, then lo), accumulating all of
           hi+lo+chunks into one PSUM (8, n) region -- no DVE adds.
  stage 1.5: per N-slice copy PSUM -> SBUF with bf16 cast, split
           across DVE/Scalar halves (8-partition ops are lane-bound).
  stage 2: 11 bf16 PE transposes of (8, <=120) blocks into a single
           shared PSUM tile (120, 88) bf16, one DVE copy to SBUF.
  stage 3: one bf16 matmul lhsT=blockdiag(M)/2^16 (scale folded into
           M) -> PSUM fp32 (120, 88) = the output layout; DMA out.
x is host-repacked to a (128, 5120) fp8 SBUF image, N-slab major, so
each slab DMA is contiguous 1920B/partition lines; slab 0 is split
across both DMA queues so stage 1 starts ~0.7us after flow begins.
"""

import os
import sys

for _p in (
    "/root/.axon_site",
    "/root/.axon_site/_ro/trn_rl_repo",
    "/root/.axon_site/_ro/pypackages",
):
    if os.path.isdir(_p) and _p not in sys.path:
        sys.path.insert(0, _p)

from contextlib import ExitStack

import ml_dtypes
import numpy as np

B, L, A, C = 512, 512, 20, 8
N_REST = 8
NCORES = 8
BS = B // NCORES          # 64 batches per core
NCH = L // 128            # 4 contraction chunks of 128
BBLK = 6                  # batches per stage-2/3 block (6*20 = 120 parts)
NBLK = (BS + BBLK - 1) // BBLK   # 11
NTOT = BS * A             # 1280
SCALE = 2.0 ** 16         # fp8 T pre-scale (T ~2e-3 underflows e4m3)
NSL = [(0, 480), (480, 480), (960, 320)]   # N-slices (PSUM <=512 cols)

_CACHE = {}


def _build_M(lpm, pm):
    """M = I + V (float64), out = x @ M along the amino-acid axis."""
    lpm = lpm.astype(np.float64)
    pm = pm.astype(np.float64)
    prod = np.clip(lpm, 1e-3, 1.0) * pm
    i = np.arange(A)[:, None]
    k = np.arange(A)[None, :]
    V = np.where(k > i, prod, np.where(k < i, prod.T, 0.0))
    V[:, A - 1] = 0.0
    return np.eye(A) + V


def _build_T(w_first, w_rest):
    """Fold the 9 conv(pad=1,k=3)+avgpool(2) stages into T (512, 8), f64."""
    H = np.eye(L, dtype=np.float64)[:, None, :]        # (512, 1, 512)

    def conv(H, w):
        Hp = np.pad(H, ((0, 0), (0, 0), (1, 1)))
        sh = np.stack([Hp[:, :, t:t + H.shape[2]] for t in range(3)], axis=-1)
        return np.einsum("rcpt,oct->rop", sh, w.astype(np.float64), optimize=True)

    H = conv(H, w_first)
    H = H.reshape(H.shape[0], H.shape[1], -1, 2).mean(-1)
    for li in range(N_REST):
        H = conv(H, w_rest[li])
        H = H.reshape(H.shape[0], H.shape[1], -1, 2).mean(-1)
    return H[:, :, 0]                                   # (512, 8)


def _build_bass():
    import concourse.bacc as bacc
    import concourse.mybir as mybir
    import concourse.tile as tile

    DR = mybir.MatmulPerfMode.DoubleRow
    f32 = mybir.dt.float32
    bf16 = mybir.dt.bfloat16
    fp8 = mybir.dt.float8e4

    nc = bacc.Bacc("TRN2", target_bir_lowering=False, debug=False,
                   num_devices=1)
    # x image: (128, 64 + 5120) fp8.  Cols 0:64 carry tsp ([q][i][m16],
    # m16 = Th chunk 2q+i || Tl chunk 2q+i -- one DoubleRow matmul per
    # (slice, pair) computes hi AND lo into PSUM partitions 0:8 / 8:16);
    # riding in front of slab 0a it lands under the same semaphore with
    # zero extra descriptors.  Cols 64+ are the x slabs, slab-major;
    # slab j holds (q, i, n_j) per partition (chunk c = 2q+i), so slab
    # DMAs are contiguous partition lines.
    xr = nc.dram_tensor("xr", [128, 64 + 4 * NTOT], fp8,
                        kind="ExternalInput").ap()
    # mbdid: bf16 (120, 128): cols 0:120 = kron(I6, M)/SCALE, cols
    # 120:128 rows 0:16 = [I8; I8] -- stage 2 is a plain K=16 matmul
    # lhsT=gt block (16, 120), rhs=[I8;I8]: transposes the block AND
    # collapses hi+lo in one instruction with fp32 PSUM accumulation.
    mbdid = nc.dram_tensor("mbdid", [BBLK * A, BBLK * A + C],
                           bf16, kind="ExternalInput").ap()
    out = nc.dram_tensor("out", [BBLK * A, NBLK * C], f32,
                         kind="ExternalOutput").ap()

    with ExitStack() as ctx:
        tc = ctx.enter_context(tile.TileContext(nc))
        consts = ctx.enter_context(tc.tile_pool(name="consts", bufs=1))
        xpool = ctx.enter_context(tc.tile_pool(name="xpool", bufs=1))
        gtpool = ctx.enter_context(tc.tile_pool(name="gtpool", bufs=1))
        ps1 = ctx.enter_context(tc.tile_pool(name="ps1", bufs=1, space="PSUM"))
        ps2 = ctx.enter_context(tc.tile_pool(name="ps2", bufs=1, space="PSUM"))
        ps3 = ctx.enter_context(tc.tile_pool(name="ps3", bufs=1, space="PSUM"))

        # -------- data in --------
        x_sb = xpool.tile([128, 64 + 4 * NTOT], fp8, name="x_sb")
        mbdid_sb = consts.tile([BBLK * A, BBLK * A + C], bf16)
        # per-partition byte layout: [tsp 64][slab0 2*960][slab1 2*960]
        # [slab2 2*640], each slab split at its chunk-pair (q) boundary
        # so each matmul's semaphore wait covers exactly its own bytes.
        # sync carries the q0 pieces (+tsp), scalar the q1 pieces; PE
        # consumes j0, j2, j1 to match arrival order.
        def xdma(eng, lo, hi):
            eng.dma_start(out=x_sb[:, lo:hi], in_=xr[:, lo:hi])

        xdma(nc.sync, 0, 1024)          # tsp + slab0 q0
        xdma(nc.scalar, 1024, 1984)     # slab0 q1
        xdma(nc.sync, 3904, 5184)      # slab2 (whole)
        xdma(nc.scalar, 1984, 3904)    # slab1 (whole)
        nc.scalar.dma_start(out=mbdid_sb, in_=mbdid)
        mbd_sb = mbdid_sb[:, 0:BBLK * A]
        ii_sb = mbdid_sb[0:2 * C, BBLK * A:BBLK * A + C]   # [I8; I8]

        def w_ap(q):
            # stationary (128, 2, 16) fp8: [i][Th chunk 2q+i || Tl]
            v = x_sb[:, 0:64].rearrange("p (q i m) -> p q i m", q=2, i=2)
            return v[:, q]

        def x_ap(j, q, o, n):
            # moving (128, 2, n) fp8: slab j, chunk pair q, cols o:o+n
            base = 64 + 4 * o
            v = x_sb[:, base:base + 4 * n].rearrange(
                "p (q i n) -> p q i n", q=2, i=2)
            return v[:, q]

        # -------- stage 0: PE p-state warm-up --------------------------
        # The PE ramps to full clock only after ~3us of continuous busy;
        # it would otherwise sit idle until slab 0 lands.  Run dummy
        # DoubleRow matmuls on a zeroed scratch tile so stage 1 starts
        # at full speed.  Results go to a scratch PSUM bank, never read.
        warm_sb = xpool.tile([128, 1024], mybir.dt.uint8, name="warm_sb")
        nc.vector.memset(warm_sb, 0)
        warm_ps = ps2.tile([2 * C, 480], f32, name="warm_ps")
        wv = warm_sb.bitcast(fp8).rearrange("p (i n) -> p i n", i=2)
        for _ in range(7):
            nc.tensor.matmul(warm_ps, wv[:, :, 0:16], wv[:, :, 0:480],
                             start=True, stop=True, perf_mode=DR)

        # -------- stage 1: fp8 DoubleRow, hi+lo in one pass ------------
        gt_ps = [
            ps1.tile([2 * C, n], f32, name=f"gt_ps{j}")
            for j, (_, n) in enumerate(NSL)
        ]
        JORDER = (0, 2, 1)          # match slab arrival order
        for j in JORDER:
            o, n = NSL[j]
            for q in range(2):
                nc.tensor.matmul(gt_ps[j], w_ap(q), x_ap(j, q, o, n),
                                 start=(q == 0), stop=(q == 1),
                                 perf_mode=DR)

        # -------- stage 1.5: PSUM -> SBUF bf16, split across engines ----
        # gt_sb (16, 1320): rows 0:8 = hi, 8:16 = lo (collapsed before
        # stage 3).  Cols 1280:1320 zeroed so the last transpose is a
        # full 120-col block (its tail rows land as 0).
        gt_sb = gtpool.tile([2 * C, NBLK * BBLK * A], bf16)
        nc.vector.memset(gt_sb[:, NTOT:NBLK * BBLK * A], 0.0)
        for j in JORDER:
            o, n = NSL[j]
            h = n // 2
            nc.vector.tensor_copy(gt_sb[:, o:o + h], gt_ps[j][:, 0:h])
            nc.scalar.copy(gt_sb[:, o + h:o + n], gt_ps[j][:, h:n])

        # -------- stage 2: K=16 matmuls transpose AND collapse hi+lo ---
        t3_ps = ps2.tile([BBLK * A, NBLK * C], f32, name="t3_ps")
        TORDER = (0, 1, 2, 3, 8, 9, 10, 4, 5, 6, 7)   # slice 0, 2, 1
        for t in TORDER:
            o0 = t * BBLK * A
            nc.tensor.matmul(t3_ps[:, C * t:C * (t + 1)],
                             gt_sb[:, o0:o0 + BBLK * A], ii_sb,
                             start=True, stop=True)
        t3s = gtpool.tile([BBLK * A, NBLK * C], bf16)
        nc.vector.tensor_copy(t3s, t3_ps)

        # -------- stage 3: one bf16 matmul folds M (and 1/SCALE) -------
        o_ps = ps3.tile([BBLK * A, NBLK * C], f32, name="o_ps")
        nc.tensor.matmul(o_ps, mbd_sb, t3s, start=True, stop=True)
        # split the PSUM drain and out DMA across engines/queues: halves
        # the serial issue cost and overlaps the two flows.
        o_sb = gtpool.tile([BBLK * A, NBLK * C], f32)
        nc.vector.tensor_copy(o_sb[0:64, :], o_ps[0:64, :])
        nc.scalar.copy(o_sb[64:120, :], o_ps[64:120, :])
        nc.sync.dma_start(out=out[0:64, :], in_=o_sb[0:64, :])
        nc.scalar.dma_start(out=out[64:120, :], in_=o_sb[64:120, :])
    nc.compile()
    return nc


def _get_compiled():
    if "nc" not in _CACHE:
        _CACHE["nc"] = _build_bass()
    return _CACHE["nc"]


def _prep_weights(lpm, pm, w_first, w_rest):
    M = _build_M(lpm, pm)
    T = _build_T(w_first, w_rest)
    Ts = (T * SCALE).astype(np.float32)                 # (512, 8)
    Th = Ts.astype(ml_dtypes.float8_e4m3fn)
    Tl = (Ts - Th.astype(np.float32)).astype(ml_dtypes.float8_e4m3fn)
    # tsp[p, (q, i, m16)]: chunk c = 2q+i rows of [Th || Tl]
    tspack = np.zeros((2, 2, 128, 2 * C), dtype=ml_dtypes.float8_e4m3fn)
    Th4 = Th.reshape(NCH, 128, C)
    Tl4 = Tl.reshape(NCH, 128, C)
    for q in range(2):
        for i in range(2):
            tspack[q, i, :, 0:C] = Th4[2 * q + i]
            tspack[q, i, :, C:2 * C] = Tl4[2 * q + i]
    tsp = np.ascontiguousarray(
        tspack.transpose(2, 0, 1, 3)).reshape(128, 64)
    mbdid = np.zeros((BBLK * A, BBLK * A + C), np.float32)
    mbdid[:, 0:BBLK * A] = np.kron(np.eye(BBLK), M.astype(np.float32) / SCALE)
    mbdid[0:C, BBLK * A:] = np.eye(C, dtype=np.float32)
    mbdid[C:2 * C, BBLK * A:] = np.eye(C, dtype=np.float32)
    return tsp, mbdid.astype(ml_dtypes.bfloat16)


def _pack_x(xs, tsp):
    """xs (64, 512, 20) 0/1 -> fp8 image (128, 64 + 5120), tsp first."""
    xp = np.ascontiguousarray(xs.transpose(1, 0, 2)).reshape(L, NTOT)
    x8 = xp.astype(ml_dtypes.float8_e4m3fn)                  # (512, 1280)
    xc = x8.reshape(NCH, 128, NTOT)                          # (c, p, col)
    slabs = [tsp]
    for o, n in NSL:
        # (c, 128, n) -> (128, c, n) with c ordered (q, i)
        slabs.append(xc[:, :, o:o + n].transpose(1, 0, 2).reshape(128, 4 * n))
    return np.ascontiguousarray(np.concatenate(slabs, axis=1))


def _in_maps(inputs):
    x = np.asarray(inputs["x"], dtype=np.float32)       # (512, 512, 20)
    tsp, mbdid = _prep_weights(np.asarray(inputs["lpm"]),
                               np.asarray(inputs["pm"]),
                               np.asarray(inputs["w_first"]),
                               np.asarray(inputs["w_rest"]))
    in_maps = []
    for core in range(NCORES):
        xs = x[core * BS:(core + 1) * BS]               # (64, 512, 20)
        in_maps.append({
            "xr": _pack_x(xs, tsp),
            "mbdid": mbdid,
        })
    return in_maps


def _unshuffle(dev_outs):
    """dev_out[(bh, k), (t, c)] -> out[6t + bh, k, c] per core, then stack."""
    full = np.empty((B, A, C), np.float32)
    for core, d in enumerate(dev_outs):
        d = d.reshape(BBLK, A, NBLK, C)               # (bh, k, t, c)
        o = d.transpose(2, 0, 1, 3).reshape(NBLK * BBLK, A, C)
        full[core * BS:(core + 1) * BS] = o[:BS]
    return full


def _enable_jax_cache():
    try:
        import jax

        jax.config.update("jax_compilation_cache_dir", "/tmp/jax_comp_cache")
        jax.config.update("jax_persistent_cache_min_compile_time_secs", 0.0)
        jax.config.update("jax_persistent_cache_min_entry_size_bytes", 0)
    except Exception:
        pass


def _install_neff_cache():
    """Memoize the walrus compile on the (deterministic) BIR bytes so a
    fresh process reuses the NEFF instead of recompiling for minutes."""
    import hashlib
    import shutil

    import concourse.bass_utils as bu

    if getattr(bu, "_neff_cache_installed", False):
        return
    orig = bu.compile_bir_kernel
    cache_dir = "/tmp/bass_neff_cache"

    def cached(bir_json, tmpdir, neff_name="file.neff"):
        h = hashlib.sha256(bir_json).hexdigest()[:32]
        os.makedirs(cache_dir, exist_ok=True)
        cpath = os.path.join(cache_dir, f"{h}_{neff_name}")
        dst = os.path.join(tmpdir, neff_name)
        if os.path.exists(cpath):
            shutil.copyfile(cpath, dst)
            return dst
        neff = orig(bir_json, tmpdir, neff_name=neff_name)
        try:
            shutil.copyfile(neff, cpath)
        except OSError:
            pass
        return neff

    bu.compile_bir_kernel = cached
    bu._neff_cache_installed = True
    try:
        import concourse.bass2jax as b2j

        b2j.compile_bir_kernel = cached
    except Exception:
        pass


def kernel(**inputs):
    from concourse.bass_utils import run_bass_kernel_spmd

    _enable_jax_cache()
    _install_neff_cache()
    nc = _get_compiled()
    res = run_bass_kernel_spmd(nc, _in_maps(inputs), list(range(NCORES)))
    return _unshuffle([res.results[i]["out"] for i in range(NCORES)])


if __name__ == "__main__":
    rng = np.random.default_rng(0)
    demo = {
        "x": np.eye(A, dtype=np.float32)[rng.integers(0, A, (B, L))],
        "masks": np.ones((B, L), np.float32),
        "lpm": rng.standard_normal((A, A)).astype(np.float32),
        "pm": rng.random((A, A)).astype(np.float32),
        "w_first": rng.standard_normal((C, 1, 3)).astype(np.float32) * 0.3,
        "w_rest": rng.standard_normal((N_REST, C, C, 3)).astype(np.float32) * 0.2,
    }
    out = kernel(**demo)
    print("kernel output", out.shape, out.dtype)


# revision 34
# speedup vs baseline: 1.0264x; 1.0264x over previous
"""Trainium2 Bass kernel for nn_CNN_84241488544497.

The reference network collapses algebraically:
  - `_row` is identically zero (exp(-d^2/2e-4) underflows to 0.0 in fp32).
  - x is an exact 0/1 one-hot, so nz == xp and the `_column` scatter is
    xp_new = x @ M with M = I + V, V a 20x20 matrix built from lpm/pm.
  - The 9 conv+avgpool stages form one linear map T (512x8) per row.
  => out[b] = M^T @ (x[b]^T @ T)  with M (20,20), T (512,8) host-folded.

Device kernel (per core, 64 batches, pure data parallel over B=512):
  stage 1: Gt[c, (b,i)] = sum_p T[p,c] * x[p,(b,i)] via fp8e4m3
           DoubleRow matmuls: T scaled by 2^16 and split hi/lo in fp8
           (T values ~2e-3 underflow e4m3 otherwise); x one-hot is
           exact in fp8.  Each DoubleRow instruction contracts 2 K-
           subtiles (chunk pairs for hi, then lo), accumulating all of
           hi+lo+chunks into one PSUM (8, n) region -- no DVE adds.
  stage 1.5: per N-slice copy PSUM -> SBUF with bf16 cast, split
           across DVE/Scalar halves (8-partition ops are lane-bound).
  stage 2: 11 bf16 PE transposes of (8, <=120) blocks into a single
           shared PSUM tile (120, 88) bf16, one DVE copy to SBUF.
  stage 3: one bf16 matmul lhsT=blockdiag(M)/2^16 (scale folded into
           M) -> PSUM fp32 (120, 88) = the output layout; DMA out.
x is host-repacked to a (128, 5120) fp8 SBUF image, N-slab major, so
each slab DMA is contiguous 1920B/partition lines; slab 0 is split
across both DMA queues so stage 1 starts ~0.7us after flow begins.
"""

import os
import sys

for _p in (
    "/root/.axon_site",
    "/root/.axon_site/_ro/trn_rl_repo",
    "/root/.axon_site/_ro/pypackages",
):
    if os.path.isdir(_p) and _p not in sys.path:
        sys.path.insert(0, _p)

from contextlib import ExitStack

import ml_dtypes
import numpy as np

B, L, A, C = 512, 512, 20, 8
N_REST = 8
NCORES = 8
BS = B // NCORES          # 64 batches per core
NCH = L // 128            # 4 contraction chunks of 128
BBLK = 6                  # batches per stage-2/3 block (6*20 = 120 parts)
NBLK = (BS + BBLK - 1) // BBLK   # 11
NTOT = BS * A             # 1280
SCALE = 2.0 ** 16         # fp8 T pre-scale (T ~2e-3 underflows e4m3)
NSL = [(0, 480), (480, 480), (960, 320)]   # N-slices (PSUM <=512 cols)

_CACHE = {}


def _build_M(lpm, pm):
    """M = I + V (float64), out = x @ M along the amino-acid axis."""
    lpm = lpm.astype(np.float64)
    pm = pm.astype(np.float64)
    prod = np.clip(lpm, 1e-3, 1.0) * pm
    i = np.arange(A)[:, None]
    k = np.arange(A)[None, :]
    V = np.where(k > i, prod, np.where(k < i, prod.T, 0.0))
    V[:, A - 1] = 0.0
    return np.eye(A) + V


def _build_T(w_first, w_rest):
    """Fold the 9 conv(pad=1,k=3)+avgpool(2) stages into T (512, 8), f64."""
    H = np.eye(L, dtype=np.float64)[:, None, :]        # (512, 1, 512)

    def conv(H, w):
        Hp = np.pad(H, ((0, 0), (0, 0), (1, 1)))
        sh = np.stack([Hp[:, :, t:t + H.shape[2]] for t in range(3)], axis=-1)
        return np.einsum("rcpt,oct->rop", sh, w.astype(np.float64), optimize=True)

    H = conv(H, w_first)
    H = H.reshape(H.shape[0], H.shape[1], -1, 2).mean(-1)
    for li in range(N_REST):
        H = conv(H, w_rest[li])
        H = H.reshape(H.shape[0], H.shape[1], -1, 2).mean(-1)
    return H[:, :, 0]                                   # (512, 8)


def _build_bass():
    import concourse.bacc as bacc
    import concourse.mybir as mybir
    import concourse.tile as tile

    DR = mybir.MatmulPerfMode.DoubleRow
    f32 = mybir.dt.float32
    bf16 = mybir.dt.bfloat16
    fp8 = mybir.dt.float8e4

    nc = bacc.Bacc("TRN2", target_bir_lowering=False, debug=False,
                   num_devices=1)
    # x image: (128, 64 + 5120) fp8.  Cols 0:64 carry tsp ([q][i][m16],
    # m16 = Th chunk 2q+i || Tl chunk 2q+i -- one DoubleRow matmul per
    # (slice, pair) computes hi AND lo into PSUM partitions 0:8 / 8:16);
    # riding in front of slab 0a it lands under the same semaphore with
    # zero extra descriptors.  Cols 64+ are the x slabs, slab-major;
    # slab j holds (q, i, n_j) per partition (chunk c = 2q+i), so slab
    # DMAs are contiguous partition lines.
    xr = nc.dram_tensor("xr", [128, 64 + 4 * NTOT], fp8,
                        kind="ExternalInput").ap()
    # mbdid: bf16 (120, 128): cols 0:120 = kron(I6, M)/SCALE, cols
    # 120:128 rows 0:16 = [I8; I8] -- stage 2 is a plain K=16 matmul
    # lhsT=gt block (16, 120), rhs=[I8;I8]: transposes the block AND
    # collapses hi+lo in one instruction with fp32 PSUM accumulation.
    mbdid = nc.dram_tensor("mbdid", [BBLK * A, BBLK * A + C],
                           bf16, kind="ExternalInput").ap()
    out = nc.dram_tensor("out", [BBLK * A, NBLK * C], f32,
                         kind="ExternalOutput").ap()

    with ExitStack() as ctx:
        tc = ctx.enter_context(tile.TileContext(nc))
        consts = ctx.enter_context(tc.tile_pool(name="consts", bufs=1))
        xpool = ctx.enter_context(tc.tile_pool(name="xpool", bufs=1))
        gtpool = ctx.enter_context(tc.tile_pool(name="gtpool", bufs=1))
        ps1 = ctx.enter_context(tc.tile_pool(name="ps1", bufs=1, space="PSUM"))
        ps2 = ctx.enter_context(tc.tile_pool(name="ps2", bufs=1, space="PSUM"))
        ps3 = ctx.enter_context(tc.tile_pool(name="ps3", bufs=1, space="PSUM"))

        # -------- data in --------
        x_sb = xpool.tile([128, 64 + 4 * NTOT], fp8, name="x_sb")
        mbdid_sb = consts.tile([BBLK * A, BBLK * A + C], bf16)
        # per-partition byte layout: [tsp 64][slab0 2*960][slab1 2*960]
        # [slab2 2*640], each slab split at its chunk-pair (q) boundary
        # so each matmul's semaphore wait covers exactly its own bytes.
        # sync carries the q0 pieces (+tsp), scalar the q1 pieces; PE
        # consumes j0, j2, j1 to match arrival order.
        def xdma(eng, lo, hi):
            eng.dma_start(out=x_sb[:, lo:hi], in_=xr[:, lo:hi])

        xdma(nc.sync, 0, 1024)          # tsp + slab0 q0
        xdma(nc.scalar, 1024, 1984)     # slab0 q1
        xdma(nc.sync, 3904, 4544)      # slab2 q0
        xdma(nc.scalar, 4544, 5184)    # slab2 q1
        xdma(nc.sync, 1984, 2944)      # slab1 q0
        xdma(nc.scalar, 2944, 3904)    # slab1 q1
        nc.scalar.dma_start(out=mbdid_sb, in_=mbdid)
        mbd_sb = mbdid_sb[:, 0:BBLK * A]
        ii_sb = mbdid_sb[0:2 * C, BBLK * A:BBLK * A + C]   # [I8; I8]

        def w_ap(q):
            # stationary (128, 2, 16) fp8: [i][Th chunk 2q+i || Tl]
            v = x_sb[:, 0:64].rearrange("p (q i m) -> p q i m", q=2, i=2)
            return v[:, q]

        def x_ap(j, q, o, n):
            # moving (128, 2, n) fp8: slab j, chunk pair q, cols o:o+n
            base = 64 + 4 * o
            v = x_sb[:, base:base + 4 * n].rearrange(
                "p (q i n) -> p q i n", q=2, i=2)
            return v[:, q]

        # -------- stage 1: fp8 DoubleRow, hi+lo in one pass ------------
        gt_ps = [
            ps1.tile([2 * C, n], f32, name=f"gt_ps{j}")
            for j, (_, n) in enumerate(NSL)
        ]
        JORDER = (0, 2, 1)          # match slab arrival order
        for j in JORDER:
            o, n = NSL[j]
            for q in range(2):
                nc.tensor.matmul(gt_ps[j], w_ap(q), x_ap(j, q, o, n),
                                 start=(q == 0), stop=(q == 1),
                                 perf_mode=DR)

        # -------- stage 1.5: PSUM -> SBUF bf16, split across engines ----
        # gt_sb (16, 1320): rows 0:8 = hi, 8:16 = lo (collapsed before
        # stage 3).  Cols 1280:1320 zeroed so the last transpose is a
        # full 120-col block (its tail rows land as 0).
        gt_sb = gtpool.tile([2 * C, NBLK * BBLK * A], bf16)
        nc.vector.memset(gt_sb[:, NTOT:NBLK * BBLK * A], 0.0)
        for j in JORDER:
            o, n = NSL[j]
            h = n // 2
            nc.vector.tensor_copy(gt_sb[:, o:o + h], gt_ps[j][:, 0:h])
            nc.scalar.copy(gt_sb[:, o + h:o + n], gt_ps[j][:, h:n])

        # -------- stages 2+3, pipelined in 3 block-groups --------------
        # Per group g (slice-aligned): K=16 matmuls (transpose AND
        # collapse hi+lo), t3 copy -> bf16, mm3 (blockdiag M restricts
        # mixing to within-group cols), PSUM drain, out DMA.  Groups
        # complete left to right so only the LAST (small) group's chain
        # sits on the critical path after the final transposes.
        GROUPS = (                      # (blocks, out engine/queue)
            ((0, 1, 2, 3), nc.sync),    # slice 0
            ((8, 9, 10), nc.scalar),    # slice 2
            ((4, 5, 6, 7), nc.sync),    # slice 1
        )
        t3_ps = ps2.tile([BBLK * A, NBLK * C], f32, name="t3_ps")
        o_ps = ps3.tile([BBLK * A, NBLK * C], f32, name="o_ps")
        t3s = gtpool.tile([BBLK * A, NBLK * C], bf16)
        o_sb = gtpool.tile([BBLK * A, NBLK * C], f32)
        # emit all stage-2 matmuls first per group, with each group's
        # mm3 deferred one group so the PE never stalls on a DVE copy
        pend = []

        def flush():
            if not pend:
                return
            lo, hi, eng = pend.pop()
            nc.tensor.matmul(o_ps[:, lo:hi], mbd_sb, t3s[:, lo:hi],
                             start=True, stop=True)
            nc.vector.tensor_copy(o_sb[:, lo:hi], o_ps[:, lo:hi])
            eng.dma_start(out=out[:, lo:hi], in_=o_sb[:, lo:hi])

        for blocks, eng in GROUPS:
            for t in blocks:
                o0 = t * BBLK * A
                nc.tensor.matmul(t3_ps[:, C * t:C * (t + 1)],
                                 gt_sb[:, o0:o0 + BBLK * A], ii_sb,
                                 start=True, stop=True)
            lo, hi = C * blocks[0], C * (blocks[-1] + 1)
            flush()
            nc.vector.tensor_copy(t3s[:, lo:hi], t3_ps[:, lo:hi])
            pend.append((lo, hi, eng))
        flush()
    nc.compile()
    return nc


def _get_compiled():
    if "nc" not in _CACHE:
        _CACHE["nc"] = _build_bass()
    return _CACHE["nc"]


def _prep_weights(lpm, pm, w_first, w_rest):
    M = _build_M(lpm, pm)
    T = _build_T(w_first, w_rest)
    Ts = (T * SCALE).astype(np.float32)                 # (512, 8)
    Th = Ts.astype(ml_dtypes.float8_e4m3fn)
    Tl = (Ts - Th.astype(np.float32)).astype(ml_dtypes.float8_e4m3fn)
    # tsp[p, (q, i, m16)]: chunk c = 2q+i rows of [Th || Tl]
    tspack = np.zeros((2, 2, 128, 2 * C), dtype=ml_dtypes.float8_e4m3fn)
    Th4 = Th.reshape(NCH, 128, C)
    Tl4 = Tl.reshape(NCH, 128, C)
    for q in range(2):
        for i in range(2):
            tspack[q, i, :, 0:C] = Th4[2 * q + i]
            tspack[q, i, :, C:2 * C] = Tl4[2 * q + i]
    tsp = np.ascontiguousarray(
        tspack.transpose(2, 0, 1, 3)).reshape(128, 64)
    mbdid = np.zeros((BBLK * A, BBLK * A + C), np.float32)
    mbdid[:, 0:BBLK * A] = np.kron(np.eye(BBLK), M.astype(np.float32) / SCALE)
    mbdid[0:C, BBLK * A:] = np.eye(C, dtype=np.float32)
    mbdid[C:2 * C, BBLK * A:] = np.eye(C, dtype=np.float32)
    return tsp, mbdid.astype(ml_dtypes.bfloat16)


def _pack_x(xs, tsp):
    """xs (64, 512, 20) 0/1 -> fp8 image (128, 64 + 5120), tsp first."""
    xp = np.ascontiguousarray(xs.transpose(1, 0, 2)).reshape(L, NTOT)
    x8 = xp.astype(ml_dtypes.float8_e4m3fn)                  # (512, 1280)
    xc = x8.reshape(NCH, 128, NTOT)                          # (c, p, col)
    slabs = [tsp]
    for o, n in NSL:
        # (c, 128, n) -> (128, c, n) with c ordered (q, i)
        slabs.append(xc[:, :, o:o + n].transpose(1, 0, 2).reshape(128, 4 * n))
    return np.ascontiguousarray(np.concatenate(slabs, axis=1))


def _in_maps(inputs):
    x = np.asarray(inputs["x"], dtype=np.float32)       # (512, 512, 20)
    tsp, mbdid = _prep_weights(np.asarray(inputs["lpm"]),
                               np.asarray(inputs["pm"]),
                               np.asarray(inputs["w_first"]),
                               np.asarray(inputs["w_rest"]))
    in_maps = []
    for core in range(NCORES):
        xs = x[core * BS:(core + 1) * BS]               # (64, 512, 20)
        in_maps.append({
            "xr": _pack_x(xs, tsp),
            "mbdid": mbdid,
        })
    return in_maps


def _unshuffle(dev_outs):
    """dev_out[(bh, k), (t, c)] -> out[6t + bh, k, c] per core, then stack."""
    full = np.empty((B, A, C), np.float32)
    for core, d in enumerate(dev_outs):
        d = d.reshape(BBLK, A, NBLK, C)               # (bh, k, t, c)
        o = d.transpose(2, 0, 1, 3).reshape(NBLK * BBLK, A, C)
        full[core * BS:(core + 1) * BS] = o[:BS]
    return full


def _enable_jax_cache():
    try:
        import jax

        jax.config.update("jax_compilation_cache_dir", "/tmp/jax_comp_cache")
        jax.config.update("jax_persistent_cache_min_compile_time_secs", 0.0)
        jax.config.update("jax_persistent_cache_min_entry_size_bytes", 0)
    except Exception:
        pass


def _install_neff_cache():
    """Memoize the walrus compile on the (deterministic) BIR bytes so a
    fresh process reuses the NEFF instead of recompiling for minutes."""
    import hashlib
    import shutil

    import concourse.bass_utils as bu

    if getattr(bu, "_neff_cache_installed", False):
        return
    orig = bu.compile_bir_kernel
    cache_dir = "/tmp/bass_neff_cache"

    def cached(bir_json, tmpdir, neff_name="file.neff"):
        h = hashlib.sha256(bir_json).hexdigest()[:32]
        os.makedirs(cache_dir, exist_ok=True)
        cpath = os.path.join(cache_dir, f"{h}_{neff_name}")
        dst = os.path.join(tmpdir, neff_name)
        if os.path.exists(cpath):
            shutil.copyfile(cpath, dst)
            return dst
        neff = orig(bir_json, tmpdir, neff_name=neff_name)
        try:
            shutil.copyfile(neff, cpath)
        except OSError:
            pass
        return neff

    bu.compile_bir_kernel = cached
    bu._neff_cache_installed = True
    try:
        import concourse.bass2jax as b2j

        b2j.compile_bir_kernel = cached
    except Exception:
        pass


def kernel(**inputs):
    from concourse.bass_utils import run_bass_kernel_spmd

    _enable_jax_cache()
    _install_neff_cache()
    nc = _get_compiled()
    res = run_bass_kernel_spmd(nc, _in_maps(inputs), list(range(NCORES)))
    return _unshuffle([res.results[i]["out"] for i in range(NCORES)])


if __name__ == "__main__":
    rng = np.random.default_rng(0)
    demo = {
        "x": np.eye(A, dtype=np.float32)[rng.integers(0, A, (B, L))],
        "masks": np.ones((B, L), np.float32),
        "lpm": rng.standard_normal((A, A)).astype(np.float32),
        "pm": rng.random((A, A)).astype(np.float32),
        "w_first": rng.standard_normal((C, 1, 3)).astype(np.float32) * 0.3,
        "w_rest": rng.standard_normal((N_REST, C, C, 3)).astype(np.float32) * 0.2,
    }
    out = kernel(**demo)
    print("kernel output", out.shape, out.dtype)


# revision 44
# speedup vs baseline: 1.0280x; 1.0016x over previous
"""Trainium2 Bass kernel for nn_CNN_84241488544497.

The reference network collapses algebraically:
  - `_row` is identically zero (exp(-d^2/2e-4) underflows to 0.0 in fp32).
  - x is an exact 0/1 one-hot, so nz == xp and the `_column` scatter is
    xp_new = x @ M with M = I + V, V a 20x20 matrix built from lpm/pm.
  - The 9 conv+avgpool stages form one linear map T (512x8) per row.
  => out[b] = M^T @ (x[b]^T @ T)  with M (20,20), T (512,8) host-folded.

Device kernel (per core, 64 batches, pure data parallel over B=512):
  stage 1: Gt[(hl,c), (b,i)] = sum_p T[p,c] * x[p,(b,i)] via fp8e4m3
           DoubleRow matmuls.  T is scaled by 2^16 (its ~2e-3 entries
           underflow e4m3) and split hi/lo; the stationary packs
           [Th || Tl] side by side (16 cols), so ONE DoubleRow matmul
           per (N-slice, chunk pair) contracts K=256 positions and
           produces hi and lo sums at PSUM partitions 0:8 / 8:16.
           x streams through the PE exactly once (SBUF-feed floor).
  stage 1.5: per N-slice copy PSUM -> SBUF bf16 (16, n), halves split
           across DVE/ACT (few-partition ops are lane-bound).
  stage 2: 11 plain bf16 matmuls lhsT=gt block (16, 120), rhs=[I8;I8]:
           each transposes a block AND collapses hi+lo into fp32 PSUM
           (120, 88); one DVE copy to SBUF bf16.
  stage 3: one bf16 matmul lhsT=kron(I6, M)/2^16 (fp8 scale folded in)
           -> PSUM fp32 (120, 88) = the output layout; single DMA out.
x is host-repacked to a (128, 64+5120) fp8 image: 64B of packed T
weights ride in front so they land under slab 0's semaphore, then the
x slabs (N-major, chunk-pair-split across both DMA queues) so stage 1
streams behind the DMAs with per-matmul-granular semaphore waits.
Measured ~19.2-19.7us vs the 29.4us session baseline (~8.5us of which
is fixed NEFF entry/exit framework overhead).
"""

import os
import sys

for _p in (
    "/root/.axon_site",
    "/root/.axon_site/_ro/trn_rl_repo",
    "/root/.axon_site/_ro/pypackages",
):
    if os.path.isdir(_p) and _p not in sys.path:
        sys.path.insert(0, _p)

from contextlib import ExitStack

import ml_dtypes
import numpy as np

B, L, A, C = 512, 512, 20, 8
N_REST = 8
NCORES = 8
BS = B // NCORES          # 64 batches per core
NCH = L // 128            # 4 contraction chunks of 128
BBLK = 6                  # batches per stage-2/3 block (6*20 = 120 parts)
NBLK = (BS + BBLK - 1) // BBLK   # 11
NTOT = BS * A             # 1280
SCALE = 2.0 ** 16         # fp8 T pre-scale (T ~2e-3 underflows e4m3)
# N-slices (PSUM <=512 cols, boundaries 120-aligned).  Variant "c"
# (default) is the measured-fastest configuration; "a"/"b"/"d" are
# kept for A/B experiments via KERNEL_V.
KVAR = os.environ.get("KERNEL_V", "c")
if KVAR == "b":
    NSL = [(0, 480), (480, 480), (960, 240), (1200, 80)]
else:
    NSL = [(0, 480), (480, 480), (960, 320)]

_CACHE = {}


def _build_M(lpm, pm):
    """M = I + V (float64), out = x @ M along the amino-acid axis."""
    lpm = lpm.astype(np.float64)
    pm = pm.astype(np.float64)
    prod = np.clip(lpm, 1e-3, 1.0) * pm
    i = np.arange(A)[:, None]
    k = np.arange(A)[None, :]
    V = np.where(k > i, prod, np.where(k < i, prod.T, 0.0))
    V[:, A - 1] = 0.0
    return np.eye(A) + V


def _build_T(w_first, w_rest):
    """Fold the 9 conv(pad=1,k=3)+avgpool(2) stages into T (512, 8), f64."""
    H = np.eye(L, dtype=np.float64)[:, None, :]        # (512, 1, 512)

    def conv(H, w):
        Hp = np.pad(H, ((0, 0), (0, 0), (1, 1)))
        sh = np.stack([Hp[:, :, t:t + H.shape[2]] for t in range(3)], axis=-1)
        return np.einsum("rcpt,oct->rop", sh, w.astype(np.float64), optimize=True)

    H = conv(H, w_first)
    H = H.reshape(H.shape[0], H.shape[1], -1, 2).mean(-1)
    for li in range(N_REST):
        H = conv(H, w_rest[li])
        H = H.reshape(H.shape[0], H.shape[1], -1, 2).mean(-1)
    return H[:, :, 0]                                   # (512, 8)


def _build_bass():
    import concourse.bacc as bacc
    import concourse.mybir as mybir
    import concourse.tile as tile

    DR = mybir.MatmulPerfMode.DoubleRow
    f32 = mybir.dt.float32
    bf16 = mybir.dt.bfloat16
    fp8 = mybir.dt.float8e4

    nc = bacc.Bacc("TRN2", target_bir_lowering=False, debug=False,
                   num_devices=1)
    # x image: (128, 64 + 5120) fp8.  Cols 0:64 carry tsp ([q][i][m16],
    # m16 = Th chunk 2q+i || Tl chunk 2q+i -- one DoubleRow matmul per
    # (slice, pair) computes hi AND lo into PSUM partitions 0:8 / 8:16);
    # riding in front of slab 0a it lands under the same semaphore with
    # zero extra descriptors.  Cols 64+ are the x slabs, slab-major;
    # slab j holds (q, i, n_j) per partition (chunk c = 2q+i), so slab
    # DMAs are contiguous partition lines.
    xr = nc.dram_tensor("xr", [128, 64 + 4 * NTOT], fp8,
                        kind="ExternalInput").ap()
    # mbdid: bf16 (120, 128): cols 0:120 = kron(I6, M)/SCALE, cols
    # 120:128 rows 0:16 = [I8; I8] -- stage 2 is a plain K=16 matmul
    # lhsT=gt block (16, 120), rhs=[I8;I8]: transposes the block AND
    # collapses hi+lo in one instruction with fp32 PSUM accumulation.
    mbdid = nc.dram_tensor("mbdid", [BBLK * A, BBLK * A + C],
                           bf16, kind="ExternalInput").ap()
    out = nc.dram_tensor("out", [BBLK * A, NBLK * C], f32,
                         kind="ExternalOutput").ap()

    with ExitStack() as ctx:
        tc = ctx.enter_context(tile.TileContext(nc))
        consts = ctx.enter_context(tc.tile_pool(name="consts", bufs=1))
        xpool = ctx.enter_context(tc.tile_pool(name="xpool", bufs=1))
        gtpool = ctx.enter_context(tc.tile_pool(name="gtpool", bufs=1))
        ps1 = ctx.enter_context(tc.tile_pool(name="ps1", bufs=1, space="PSUM"))
        ps2 = ctx.enter_context(tc.tile_pool(name="ps2", bufs=1, space="PSUM"))
        ps3 = ctx.enter_context(tc.tile_pool(name="ps3", bufs=1, space="PSUM"))

        # -------- data in --------
        x_sb = xpool.tile([128, 64 + 4 * NTOT], fp8, name="x_sb")
        mbdid_sb = consts.tile([BBLK * A, BBLK * A + C], bf16)
        # per-partition byte layout: [tsp 64][slab0 2*960][slab1 2*960]
        # [slab2 2*640], each slab split at its chunk-pair (q) boundary
        # so each matmul's semaphore wait covers exactly its own bytes.
        # sync carries the q0 pieces (+tsp), scalar the q1 pieces; PE
        # consumes j0, j2, j1 to match arrival order.
        def xdma(eng, lo, hi):
            eng.dma_start(out=x_sb[:, lo:hi], in_=xr[:, lo:hi])

        if KVAR == "b":
            # slab order = slice order; q0 pieces on sync, q1 on scalar
            for j, (o, n) in enumerate(NSL):
                base = 64 + 4 * o
                lo0 = 0 if j == 0 else base          # fold tsp into s0q0
                xdma(nc.sync, lo0, base + 2 * n)
                xdma(nc.scalar, base + 2 * n, base + 4 * n)
            JORDER = tuple(range(len(NSL)))
        elif KVAR == "d":
            # 4 x-DMAs: slab0 q-split for the earliest start, slab2 and
            # slab1 whole on opposite queues (2 fewer semaphores to
            # verify/clear in the framework exit sequence)
            xdma(nc.sync, 0, 1024)          # tsp + slab0 q0
            xdma(nc.scalar, 1024, 1984)     # slab0 q1
            xdma(nc.sync, 3904, 5184)      # slab2 whole
            xdma(nc.scalar, 1984, 3904)    # slab1 whole
            JORDER = (0, 2, 1)
        else:
            xdma(nc.sync, 0, 1024)          # tsp + slab0 q0
            xdma(nc.scalar, 1024, 1984)     # slab0 q1
            xdma(nc.sync, 3904, 4544)      # slab2 q0
            xdma(nc.scalar, 4544, 5184)    # slab2 q1
            xdma(nc.sync, 1984, 2944)      # slab1 q0
            xdma(nc.scalar, 2944, 3904)    # slab1 q1
            JORDER = (0, 2, 1)              # match slab arrival order
        nc.scalar.dma_start(out=mbdid_sb, in_=mbdid)
        mbd_sb = mbdid_sb[:, 0:BBLK * A]
        ii_sb = mbdid_sb[0:2 * C, BBLK * A:BBLK * A + C]   # [I8; I8]

        def w_ap(q):
            # stationary (128, 2, 16) fp8: [i][Th chunk 2q+i || Tl]
            v = x_sb[:, 0:64].rearrange("p (q i m) -> p q i m", q=2, i=2)
            return v[:, q]

        def x_ap(j, q, o, n):
            # moving (128, 2, n) fp8: slab j, chunk pair q, cols o:o+n
            base = 64 + 4 * o
            v = x_sb[:, base:base + 4 * n].rearrange(
                "p (q i n) -> p q i n", q=2, i=2)
            return v[:, q]

        # -------- stage 1: fp8 DoubleRow, hi+lo in one pass ------------
        gt_ps = [
            ps1.tile([2 * C, n], f32, name=f"gt_ps{j}")
            for j, (_, n) in enumerate(NSL)
        ]

        def mm1(j):
            o, n = NSL[j]
            for q in range(2):
                nc.tensor.matmul(gt_ps[j], w_ap(q), x_ap(j, q, o, n),
                                 start=(q == 0), stop=(q == 1),
                                 perf_mode=DR)

        # stage 1.5 copies: PSUM -> SBUF bf16, split across engines.
        # gt_sb (16, 1320): rows 0:8 = hi, 8:16 = lo (collapsed by the
        # stage-2 matmul).  Cols 1280:1320 zeroed so the last block is a
        # full 120-col transpose (its tail rows land as 0).
        gt_sb = gtpool.tile([2 * C, NBLK * BBLK * A], bf16)
        nc.vector.memset(gt_sb[:, NTOT:NBLK * BBLK * A], 0.0)

        def gtcopy(j):
            o, n = NSL[j]
            if n <= 120:
                nc.vector.tensor_copy(gt_sb[:, o:o + n], gt_ps[j])
                return
            h = n // 2
            nc.vector.tensor_copy(gt_sb[:, o:o + h], gt_ps[j][:, 0:h])
            nc.scalar.copy(gt_sb[:, o + h:o + n], gt_ps[j][:, h:n])

        # stage 2: K=16 matmuls transpose AND collapse hi+lo
        t3_ps = ps2.tile([BBLK * A, NBLK * C], f32, name="t3_ps")
        t3s = gtpool.tile([BBLK * A, NBLK * C], bf16)

        def blocks_of(j):
            o, n = NSL[j]
            b1 = NBLK if o + n >= NTOT else (o + n) // (BBLK * A)
            return range(o // (BBLK * A), b1)

        def mm2(j):
            for t in blocks_of(j):
                o0 = t * BBLK * A
                nc.tensor.matmul(t3_ps[:, C * t:C * (t + 1)],
                                 gt_sb[:, o0:o0 + BBLK * A], ii_sb,
                                 start=True, stop=True)

        def t3copy(j):
            bs = blocks_of(j)
            nc.vector.tensor_copy(t3s[:, C * bs[0]:C * (bs[-1] + 1)],
                                  t3_ps[:, C * bs[0]:C * (bs[-1] + 1)])

        if KVAR == "b":
            # interleave: transposes of early slices fill PE stalls while
            # later slabs arrive; per-slice t3 copies keep only block 10
            # on the critical tail.
            mm1(0); mm1(1)
            gtcopy(0); gtcopy(1)
            mm2(0)
            mm1(2); mm1(3)
            gtcopy(2); gtcopy(3)
            mm2(1); mm2(2); mm2(3)
            for j in JORDER:
                t3copy(j)
        else:
            for j in JORDER:
                mm1(j)
            for j in JORDER:
                gtcopy(j)
            for j in JORDER:
                mm2(j)
            nc.vector.tensor_copy(t3s, t3_ps)

        # -------- stage 3: one bf16 matmul folds M (and 1/SCALE) -------
        o_ps = ps3.tile([BBLK * A, NBLK * C], f32, name="o_ps")
        nc.tensor.matmul(o_ps, mbd_sb, t3s, start=True, stop=True)
        o_sb = gtpool.tile([BBLK * A, NBLK * C], f32)
        if KVAR in ("c", "d"):
            # single drain + single out DMA: one less DMA semaphore for
            # the exit housekeeping, at the cost of a serial flow.
            nc.vector.tensor_copy(o_sb, o_ps)
            nc.sync.dma_start(out=out, in_=o_sb)
        else:
            # split the PSUM drain and out DMA across engines/queues:
            # halves the serial issue cost and overlaps the two flows.
            nc.vector.tensor_copy(o_sb[0:64, :], o_ps[0:64, :])
            nc.scalar.copy(o_sb[64:120, :], o_ps[64:120, :])
            nc.sync.dma_start(out=out[0:64, :], in_=o_sb[0:64, :])
            nc.scalar.dma_start(out=out[64:120, :], in_=o_sb[64:120, :])
    nc.compile()
    return nc


def _get_compiled():
    if "nc" not in _CACHE:
        _CACHE["nc"] = _build_bass()
    return _CACHE["nc"]


def _prep_weights(lpm, pm, w_first, w_rest):
    M = _build_M(lpm, pm)
    T = _build_T(w_first, w_rest)
    Ts = (T * SCALE).astype(np.float32)                 # (512, 8)
    Th = Ts.astype(ml_dtypes.float8_e4m3fn)
    Tl = (Ts - Th.astype(np.float32)).astype(ml_dtypes.float8_e4m3fn)
    # tsp[p, (q, i, m16)]: chunk c = 2q+i rows of [Th || Tl]
    tspack = np.zeros((2, 2, 128, 2 * C), dtype=ml_dtypes.float8_e4m3fn)
    Th4 = Th.reshape(NCH, 128, C)
    Tl4 = Tl.reshape(NCH, 128, C)
    for q in range(2):
        for i in range(2):
            tspack[q, i, :, 0:C] = Th4[2 * q + i]
            tspack[q, i, :, C:2 * C] = Tl4[2 * q + i]
    tsp = np.ascontiguousarray(
        tspack.transpose(2, 0, 1, 3)).reshape(128, 64)
    mbdid = np.zeros((BBLK * A, BBLK * A + C), np.float32)
    mbdid[:, 0:BBLK * A] = np.kron(np.eye(BBLK), M.astype(np.float32) / SCALE)
    mbdid[0:C, BBLK * A:] = np.eye(C, dtype=np.float32)
    mbdid[C:2 * C, BBLK * A:] = np.eye(C, dtype=np.float32)
    return tsp, mbdid.astype(ml_dtypes.bfloat16)


def _pack_x(xs, tsp):
    """xs (64, 512, 20) 0/1 -> fp8 image (128, 64 + 5120), tsp first."""
    xp = np.ascontiguousarray(xs.transpose(1, 0, 2)).reshape(L, NTOT)
    x8 = xp.astype(ml_dtypes.float8_e4m3fn)                  # (512, 1280)
    xc = x8.reshape(NCH, 128, NTOT)                          # (c, p, col)
    slabs = [tsp]
    for o, n in NSL:
        # (c, 128, n) -> (128, c, n) with c ordered (q, i)
        slabs.append(xc[:, :, o:o + n].transpose(1, 0, 2).reshape(128, 4 * n))
    return np.ascontiguousarray(np.concatenate(slabs, axis=1))


def _in_maps(inputs):
    x = np.asarray(inputs["x"], dtype=np.float32)       # (512, 512, 20)
    tsp, mbdid = _prep_weights(np.asarray(inputs["lpm"]),
                               np.asarray(inputs["pm"]),
                               np.asarray(inputs["w_first"]),
                               np.asarray(inputs["w_rest"]))
    in_maps = []
    for core in range(NCORES):
        xs = x[core * BS:(core + 1) * BS]               # (64, 512, 20)
        in_maps.append({
            "xr": _pack_x(xs, tsp),
            "mbdid": mbdid,
        })
    return in_maps


def _unshuffle(dev_outs):
    """dev_out[(bh, k), (t, c)] -> out[6t + bh, k, c] per core, then stack."""
    full = np.empty((B, A, C), np.float32)
    for core, d in enumerate(dev_outs):
        d = d.reshape(BBLK, A, NBLK, C)               # (bh, k, t, c)
        o = d.transpose(2, 0, 1, 3).reshape(NBLK * BBLK, A, C)
        full[core * BS:(core + 1) * BS] = o[:BS]
    return full


def _enable_jax_cache():
    try:
        import jax

        jax.config.update("jax_compilation_cache_dir", "/tmp/jax_comp_cache")
        jax.config.update("jax_persistent_cache_min_compile_time_secs", 0.0)
        jax.config.update("jax_persistent_cache_min_entry_size_bytes", 0)
    except Exception:
        pass


def _install_neff_cache():
    """Memoize the walrus compile on the (deterministic) BIR bytes so a
    fresh process reuses the NEFF instead of recompiling for minutes."""
    import hashlib
    import shutil

    import concourse.bass_utils as bu

    if getattr(bu, "_neff_cache_installed", False):
        return
    orig = bu.compile_bir_kernel
    cache_dir = "/tmp/bass_neff_cache"

    def cached(bir_json, tmpdir, neff_name="file.neff"):
        h = hashlib.sha256(bir_json).hexdigest()[:32]
        os.makedirs(cache_dir, exist_ok=True)
        cpath = os.path.join(cache_dir, f"{h}_{neff_name}")
        dst = os.path.join(tmpdir, neff_name)
        if os.path.exists(cpath):
            shutil.copyfile(cpath, dst)
            return dst
        neff = orig(bir_json, tmpdir, neff_name=neff_name)
        try:
            shutil.copyfile(neff, cpath)
        except OSError:
            pass
        return neff

    bu.compile_bir_kernel = cached
    bu._neff_cache_installed = True
    try:
        import concourse.bass2jax as b2j

        b2j.compile_bir_kernel = cached
    except Exception:
        pass


def kernel(**inputs):
    from concourse.bass_utils import run_bass_kernel_spmd

    _enable_jax_cache()
    _install_neff_cache()
    nc = _get_compiled()
    res = run_bass_kernel_spmd(nc, _in_maps(inputs), list(range(NCORES)))
    return _unshuffle([res.results[i]["out"] for i in range(NCORES)])


if __name__ == "__main__":
    rng = np.random.default_rng(0)
    demo = {
        "x": np.eye(A, dtype=np.float32)[rng.integers(0, A, (B, L))],
        "masks": np.ones((B, L), np.float32),
        "lpm": rng.standard_normal((A, A)).astype(np.float32),
        "pm": rng.random((A, A)).astype(np.float32),
        "w_first": rng.standard_normal((C, 1, 3)).astype(np.float32) * 0.3,
        "w_rest": rng.standard_normal((N_REST, C, C, 3)).astype(np.float32) * 0.2,
    }
    out = kernel(**demo)
    print("kernel output", out.shape, out.dtype)
